# revision 3
# baseline (speedup 1.0000x reference)
"""Trainium2 Bass kernel for nn_Debias (histogram_binning) — v3.

Strategy (data-parallel over samples, 8 cores; class-major fp16 layout):
  Host prep (per core): pred[:, 1:51] -> fp16, transposed to class-major
  [50, 124928] (class 0 never wins the argmax over 1..50, so it is not
  shipped); gt -> fp16 (0..50 exact). The 72 leftover samples per core are
  histogrammed exactly (f32 argmax) on the host and added in.

  Device, per core: sample s of chunk k lives at (partition p, free col w),
  s = O_k + p*W_k + w. Chunk widths [328, 328, 320] keep every per-partition
  DMA run at W*2B >= 512B (full DMA descriptor rate; <512B pays 2x).
    - rowmax: pairwise-max tree of fp16 tensor_tensor ops on DVE (2x mode;
      tensor_reduce has no 2x mode so the tree is 2x faster than a reduce)
    - ohp = is_equal(pred, rowmax broadcast)  fp16 TT, 2x, DVE
    - ohg[:, c, :] = is_equal(gt_f16, c)      fp16 tensor_scalar (4x on DVE);
      a slice of classes runs on GPSIMD to unload DVE (GPSIMD supports
      tensor_scalar but not tensor_tensor on this backend)
    - PE: per sample column w: psum[50,51] += ohp[:,:,w]^T @ ohg[:,:,w]
      (contraction over the 128 partitions = 128 samples per matmul)

  Host post: sum 8 local [50,51] histograms + host tail, then the tiny
  [51,51] EMA postprocess (exact mirror of the reference).
"""

import numpy as np
from contextlib import ExitStack

from concourse import tile, bacc, mybir
from concourse.bass_utils import run_bass_kernel_spmd

N_CORES = 8
C = 51                  # num classes
NC = 50                 # classes 1..50 (shipped to device)
NUM_SAMPLES = 1_000_000
S_CORE = NUM_SAMPLES // N_CORES    # 125000 samples per core
P = 128                 # SBUF partitions
W_SIZES = [328, 328, 320]          # per-partition chunk widths, sum = 976
SPP = sum(W_SIZES)                 # 976 samples per partition
MAIN = P * SPP                     # 124928 device samples per core
TAIL_H = S_CORE - MAIN             # 72 samples per core, handled on host
NSUB = 4                           # sub-blocks per chunk (eq + matmul)

# ohg engine split: classes [0, OHG_GP) on GPSIMD, [OHG_GP, 51) on DVE.
# Model: DVE TS 4x ~0.26ns/elem vs GPSIMD ~1.4ns/elem + 95ns launch;
# 33 GP classes balances DVE (tree+eq+18 cls) ~55us vs GPSIMD ~55us.
OHG_GP = 33

f32 = mybir.dt.float32
f16 = mybir.dt.float16

_CACHE = {}

ALL_PARTS = ("dma", "tree", "eq", "ohg", "pe")


def _emit_histogram(nc, tc, ctx, pred_halves, gt_chunks, hist_ap,
                    parts=ALL_PARTS):
    """Emit one full per-core histogram computation.
    pred_halves[k][h]: DRAM AP [P, 25, W_k] for chunk k, class half h.
    gt_chunks[k]: DRAM AP [P, W_k] (fp16).
    `parts` lets timing probes drop stages (data becomes garbage but the
    instruction mix/time of the remaining stages is preserved)."""
    const_pool = ctx.enter_context(tc.tile_pool(name="const", bufs=1))
    pred_pool = ctx.enter_context(tc.tile_pool(name="pred", bufs=2))
    gt_pool = ctx.enter_context(tc.tile_pool(name="gt", bufs=2))
    lvl_pool = ctx.enter_context(tc.tile_pool(name="lvl", bufs=2))
    mx_pool = ctx.enter_context(tc.tile_pool(name="mx", bufs=2))
    ohp_pool = ctx.enter_context(tc.tile_pool(name="ohp", bufs=3))
    ohg_pool = ctx.enter_context(tc.tile_pool(name="ohg", bufs=2))
    out_pool = ctx.enter_context(tc.tile_pool(name="out", bufs=1))
    psum_pool = ctx.enter_context(tc.tile_pool(name="psum", bufs=1, space="PSUM"))

    psum_t = psum_pool.tile([NC, C], f32)
    first_mm = [True]
    mm = mybir.AluOpType.max
    eq = mybir.AluOpType.is_equal

    for k, W in enumerate(W_SIZES):
        predt = pred_pool.tile([P, NC, W], f16, tag="pred")
        gtt = gt_pool.tile([P, W], f16, tag="gt")
        if "dma" in parts:
            nc.gpsimd.dma_start(gtt[:], gt_chunks[k])
            nc.sync.dma_start(predt[:, 0:25, :], pred_halves[k][0])
            nc.scalar.dma_start(predt[:, 25:50, :], pred_halves[k][1])

        # --- gt one-hot: ohg[:, c, :] = (gt == c), split GPSIMD/DVE
        ohg = ohg_pool.tile([P, C, W], f16, tag="ohg")
        if "ohg" in parts:
            for c in range(C):
                eng = nc.gpsimd if c < OHG_GP else nc.vector
                eng.tensor_scalar(ohg[:, c, :], gtt[:], float(c), None,
                                  op0=eq)

        # --- pairwise max tree over the 50 classes -> mxt [P, W]
        t25 = lvl_pool.tile([P, 25, W], f16, tag="t25")
        mxt = mx_pool.tile([P, W], f16, tag="mxt")
        if "tree" in parts:
            nc.vector.tensor_tensor(t25[:], predt[:, 0:25, :],
                                    predt[:, 25:50, :], op=mm)
            # in-place halvings inside t25 (same-position writes are safe)
            nc.vector.tensor_tensor(t25[:, 0:12, :], t25[:, 0:12, :],
                                    t25[:, 12:24, :], op=mm)
            nc.vector.tensor_tensor(t25[:, 0:6, :], t25[:, 0:6, :],
                                    t25[:, 6:12, :], op=mm)
            nc.vector.tensor_tensor(t25[:, 0:3, :], t25[:, 0:3, :],
                                    t25[:, 3:6, :], op=mm)
            nc.vector.tensor_tensor(t25[:, 0:1, :], t25[:, 0:1, :],
                                    t25[:, 1:2, :], op=mm)
            nc.vector.tensor_tensor(t25[:, 0:1, :], t25[:, 0:1, :],
                                    t25[:, 2:3, :], op=mm)
            nc.vector.tensor_tensor(mxt[:].unsqueeze(1), t25[:, 0:1, :],
                                    t25[:, 24:25, :], op=mm)

        # --- sub-blocks: ohp one-hot + matmul accumulation
        wsub = W // NSUB
        for j in range(NSUB):
            w0 = j * wsub
            ohp = ohp_pool.tile([P, NC, wsub], f16, tag="ohp")
            if "eq" in parts:
                nc.vector.tensor_tensor(
                    ohp[:], predt[:, :, w0:w0 + wsub],
                    mxt[:, w0:w0 + wsub].unsqueeze(1)
                        .broadcast_to([P, NC, wsub]),
                    op=eq)
            if "pe" in parts:
                for w in range(wsub):
                    nc.tensor.matmul(psum_t[:],
                                     lhsT=ohp[:, :, w],
                                     rhs=ohg[:, :, w0 + w],
                                     start=first_mm[0], stop=False)
                    first_mm[0] = False

    if "pe" in parts:
        # closing matmul on zeroed operands to set stop
        zt = const_pool.tile([P, NC], f16)
        zg = const_pool.tile([P, C], f16)
        nc.vector.memset(zt[:], 0.0)
        nc.vector.memset(zg[:], 0.0)
        nc.tensor.matmul(psum_t[:], lhsT=zt[:], rhs=zg[:],
                         start=False, stop=True)
        histb = out_pool.tile([NC, C], f32)
        nc.scalar.copy(histb[:], psum_t[:])
        nc.sync.dma_start(hist_ap[:], histb[:])


def _build(repeat=None, internal_io=False, parts=ALL_PARTS):
    """repeat=None: production build (external pred/gt).
    repeat=R with internal_io=True: timing build — pred/gt are internal DRAM
    scratch (no host transfer), whole computation looped R times in-NEFF."""
    if "dve" in parts:  # back-compat alias
        parts = tuple(set(parts) - {"dve"}) + ("tree", "eq", "ohg")
    nc = bacc.Bacc("TRN2", target_bir_lowering=False, debug=False,
                   num_devices=N_CORES)
    if internal_io:
        nc.dram_tensor("tick", [1], f32, kind="ExternalInput").ap()
        pred_ap = nc.dram_tensor("predt_i", [NC, MAIN], f16).ap()
        gt_ap = nc.dram_tensor("gt_i", [MAIN], f16).ap()
    else:
        pred_ap = nc.dram_tensor("predt", [NC, MAIN], f16,
                                 kind="ExternalInput").ap()
        gt_ap = nc.dram_tensor("gt", [MAIN], f16, kind="ExternalInput").ap()
    hist_ap = nc.dram_tensor("hist", [NC, C], f32, kind="ExternalOutput").ap()

    offs = [P * sum(W_SIZES[:i]) for i in range(len(W_SIZES))]
    pred_halves = []
    gt_chunks = []
    for k, W in enumerate(W_SIZES):
        halves = []
        for h in (0, 1):
            halves.append(
                pred_ap[h * 25:(h + 1) * 25, offs[k]:offs[k] + P * W]
                .rearrange("c (p w) -> p c w", p=P))
        pred_halves.append(halves)
        gt_chunks.append(
            gt_ap[offs[k]:offs[k] + P * W].rearrange("(p w) -> p w", p=P))

    with tile.TileContext(nc) as tc:
        with ExitStack() as ctx:
            if repeat is None:
                _emit_histogram(nc, tc, ctx, pred_halves, gt_chunks, hist_ap,
                                parts=parts)
            else:
                with tc.For_i(0, repeat, 1,
                              hint_engines=(mybir.EngineType.PE,
                                            mybir.EngineType.DVE)):
                    _emit_histogram(nc, tc, ctx, pred_halves, gt_chunks,
                                    hist_ap, parts=parts)
    nc.compile()
    return nc


def _get_nc():
    if "nc" not in _CACHE:
        _CACHE["nc"] = _build()
    return _CACHE["nc"]


def _device_histogram(pred: np.ndarray, gt: np.ndarray,
                      want_trace: bool = False):
    """Run the SPMD kernel; return (global [51,51] f32 histogram, results).
    Device covers the first MAIN samples of each core's shard; the caller
    adds the host-side tail."""
    nc = _get_nc()
    in_maps = []
    for i in range(N_CORES):
        sl = slice(i * S_CORE, i * S_CORE + MAIN)
        predt = np.ascontiguousarray(pred[sl, 1:C].T.astype(np.float16))
        gts = gt[sl].astype(np.float16)
        in_maps.append({"predt": predt, "gt": gts})
    res = run_bass_kernel_spmd(nc, in_maps, list(range(N_CORES)),
                               trace=want_trace)
    hist = np.zeros((C, C), dtype=np.float32)
    for r in res.results:
        hist[1:C, :] += r["hist"]
    return hist, res


def kernel(pred, rel_count, gt, istrain):
    pred = np.asarray(pred)
    rel_count = np.asarray(rel_count, dtype=np.float32)
    gt = np.asarray(gt)
    if not int(np.asarray(istrain)):
        return rel_count

    num = pred.shape[0]
    hist, _ = _device_histogram(pred, gt)

    # host tail: 72 samples per core, exact f32 argmax
    for i in range(N_CORES):
        lo = i * S_CORE + MAIN
        hi = (i + 1) * S_CORE
        pidx = np.argmax(pred[lo:hi, 1:C], axis=-1) + 1
        np.add.at(hist, (pidx, gt[lo:hi].astype(np.int64)), np.float32(1.0))

    # Small [51,51] postprocessing (exact mirror of the reference, f32).
    idx = hist.sum(axis=1, dtype=np.float32) / np.float32(num)
    gate = np.where(idx > 0.0, np.float32(0.9), np.float32(1.0))
    hist = hist.copy()
    hist[:, 0] = 0.0
    norm = hist / (hist.sum(axis=1, keepdims=True, dtype=np.float32)
                   + np.float32(1e-10))
    norm = norm.astype(np.float32)
    ema = gate[:, None] * rel_count + np.float32(0.1) * norm
    out = np.where(rel_count.sum(dtype=np.float32) == 0.0, norm, ema)
    return out.astype(np.float32)


# revision 18
# speedup vs baseline: 5.2900x; 5.2900x over previous
"""Trainium2 Bass kernel for nn_Debias (histogram_binning) — v4.

Strategy (data-parallel over samples, 8 cores; class-major fp16 layout):
  Host prep (per core): pred[:, 1:51] -> fp16, packed per (chunk, partition)
  into the exact SBUF layout [P, 50, W] so every DMA descriptor is one
  contiguous 24.4KB per-partition block (small/strided descriptors run at
  ~100GB/s on HW; >=8KB blocks reach ~340-425GB/s). Class 0 is never the
  argmax over 1..50, so it is not shipped. gt -> fp16 (0..50 exact).
  The 72 leftover samples per core are histogrammed exactly on the host.

  Device, per core: sample s of chunk k lives at (partition p, free col w).
    - rowmax: per-half pairwise-max trees of fp16 tensor_tensor ops on DVE
      (2x mode; tensor_reduce has no 2x mode, a TT tree is 2x faster)
    - ohp = is_equal(pred, rowmax broadcast)   fp16 TT, 2x, DVE
    - ohg[:, c, :] = (gt == c): DVE tensor_scalar immediate (4x mode) for
      some classes; ACT delta trick Relu(1 - Abs(gt - c)) (exact 0/1 for
      integer-valued fp16 gt) for the rest — ACT is otherwise idle.
      (GPSIMD tensor_scalar measured ~5.3us/instr on HW — unusable.)
    - PE: column pairs: psum[100,102] += lhsT[ohp pair] @ rhs[ohg pair],
      interleaved (c w) ordering; host decodes the stride-2 diagonal blocks.

  Host post: sum 8 local histograms + host tail, then the tiny [51,51]
  EMA postprocess (exact mirror of the reference).
"""

import numpy as np
from contextlib import ExitStack

from concourse import tile, bacc, mybir
from concourse.bass_utils import run_bass_kernel_spmd

N_CORES = 8
C = 51                  # num classes
NC = 50                 # classes 1..50 (shipped to device)
NUM_SAMPLES = 1_000_000
S_CORE = NUM_SAMPLES // N_CORES    # 125000 samples per core
P = 128                 # SBUF partitions
W_SIZES = [244, 244, 244, 244]     # per-partition chunk widths, sum = 976
SPP = sum(W_SIZES)                 # 976 samples per partition
MAIN = P * SPP                     # 124928 device samples per core
TAIL_H = S_CORE - MAIN             # 72 samples per core, handled on host
NSUB = 2                           # sub-blocks per chunk (eq + matmul)

# ohg engine split: classes [0, OHG_ACT) via ACT delta trick, rest via DVE
# tensor_scalar (4x). ACT 2-pass ~0.47us/class-chunk vs DVE ~0.07us.
OHG_ACT = 24
MM_PAIR = True          # 2-column matmul batching

f32 = mybir.dt.float32
f16 = mybir.dt.float16

_CACHE = {}

ALL_PARTS = ("dma", "tree", "eq", "ohg", "pe")


def _emit_setup(nc, tc, ctx):
    """One-time setup (outside any timing loop): per-class ACT bias tile."""
    setup_pool = ctx.enter_context(tc.tile_pool(name="setup", bufs=1))
    bias_t = setup_pool.tile([P, C], f32)
    for c in range(C):
        nc.gpsimd.memset(bias_t[:, c:c + 1], -float(c))
    return bias_t


def _emit_histogram(nc, tc, ctx, pred_chunks, gt_chunks, hist_ap, bias_t,
                    parts=ALL_PARTS):
    """Emit one full per-core histogram computation.
    pred_chunks[k]: DRAM AP [P, 50, W_k] (contiguous per partition).
    gt_chunks[k]: DRAM AP [P, W_k] (fp16).
    `parts` lets timing probes drop stages (data becomes garbage but the
    instruction mix/time of the remaining stages is preserved)."""
    const_pool = ctx.enter_context(tc.tile_pool(name="const", bufs=1))
    pred_pool = ctx.enter_context(tc.tile_pool(name="pred", bufs=2))
    gt_pool = ctx.enter_context(tc.tile_pool(name="gt", bufs=2))
    lvl_pool = ctx.enter_context(tc.tile_pool(name="lvl", bufs=2))
    mx_pool = ctx.enter_context(tc.tile_pool(name="mx", bufs=2))
    ohp_pool = ctx.enter_context(tc.tile_pool(name="ohp", bufs=3))
    ohg_pool = ctx.enter_context(tc.tile_pool(name="ohg", bufs=2))
    sc_pool = ctx.enter_context(tc.tile_pool(name="sc", bufs=2))
    out_pool = ctx.enter_context(tc.tile_pool(name="out", bufs=1))
    psum_pool = ctx.enter_context(tc.tile_pool(name="psum", bufs=1, space="PSUM"))

    MD = 2 if MM_PAIR else 1
    psum_t = psum_pool.tile([MD * NC, MD * C], f32)
    first_mm = [True]
    mm = mybir.AluOpType.max
    eq = mybir.AluOpType.is_equal
    Abs = mybir.ActivationFunctionType.Abs
    Relu = mybir.ActivationFunctionType.Relu

    def halftree(x25, t12):
        # max over 25 classes: x25 [P,25,W] -> root in t12[:,0:1,:]
        nc.vector.tensor_tensor(t12[:], x25[:, 0:12, :], x25[:, 12:24, :],
                                op=mm)
        nc.vector.tensor_tensor(t12[:, 0:6, :], t12[:, 0:6, :],
                                t12[:, 6:12, :], op=mm)
        nc.vector.tensor_tensor(t12[:, 0:3, :], t12[:, 0:3, :],
                                t12[:, 3:6, :], op=mm)
        nc.vector.tensor_tensor(t12[:, 0:1, :], t12[:, 0:1, :],
                                t12[:, 1:2, :], op=mm)
        nc.vector.tensor_tensor(t12[:, 0:1, :], t12[:, 0:1, :],
                                t12[:, 2:3, :], op=mm)
        nc.vector.tensor_tensor(t12[:, 0:1, :], t12[:, 0:1, :],
                                x25[:, 24:25, :], op=mm)

    need_pred = "dma" in parts or "tree" in parts or "eq" in parts
    need_gt = "dma" in parts or "ohg" in parts
    for k, W in enumerate(W_SIZES):
        if need_pred:
            predt = pred_pool.tile([P, NC, W], f16, tag="pred")
        if need_gt:
            gtt = gt_pool.tile([P, W], f16, tag="gt")
        if "dma" in parts:
            nc.gpsimd.dma_start(gtt[:], gt_chunks[k])
            nc.sync.dma_start(predt[:, 0:25, :], pred_chunks[k][:, 0:25, :])
            nc.scalar.dma_start(predt[:, 25:50, :], pred_chunks[k][:, 25:50, :])
        else:
            # timing probes: cheap producers on otherwise-idle engines
            if "tree" in parts or "eq" in parts:
                nc.gpsimd.memset(predt[:], 0)
            if "ohg" in parts:
                nc.gpsimd.memset(gtt[:], 0)

        # --- gt one-hot, pair-major: ohg[p, q, c, w2] = (gt[p, 2q+w2] == c)
        NPC = W // 2
        if "ohg" in parts or "pe" in parts:
            ohg = ohg_pool.tile([P, NPC, C, 2], f16, tag="ohg")
        if "ohg" not in parts and "pe" in parts:
            nc.gpsimd.memset(ohg[:], 0)
        if "ohg" in parts:
            gtp = gtt[:].rearrange("p (q w) -> p q w", w=2)
            for c in range(OHG_ACT):
                sc = sc_pool.tile([P, W], f16, tag="sc")
                nc.scalar.activation(sc[:], gtt[:], Abs,
                                     bias=bias_t[:, c:c + 1])
                nc.scalar.activation(
                    ohg[:, :, c, :],
                    sc[:].rearrange("p (q w) -> p q w", w=2),
                    Relu, bias=1.0, scale=-1.0)
            for c in range(OHG_ACT, C):
                nc.vector.tensor_scalar(ohg[:, :, c, :], gtp, float(c), None,
                                        op0=eq)

        # --- per-half pairwise max trees -> mxt [P, W]
        if "tree" in parts or "eq" in parts:
            mxt = mx_pool.tile([P, W], f16, tag="mxt")
        if "tree" not in parts and "eq" in parts:
            nc.gpsimd.memset(mxt[:], 0)
        if "tree" in parts:
            t12a = lvl_pool.tile([P, 12, W], f16, tag="t12a")
            t12b = lvl_pool.tile([P, 12, W], f16, tag="t12b")
            halftree(predt[:, 0:25, :], t12a)
            halftree(predt[:, 25:50, :], t12b)
            nc.vector.tensor_tensor(mxt[:].unsqueeze(1), t12a[:, 0:1, :],
                                    t12b[:, 0:1, :], op=mm)

        # --- sub-blocks: ohp one-hot (pair-major) + matmul accumulation
        wsub = W // NSUB
        npj = wsub // 2
        for j in range(NSUB):
            w0 = j * wsub
            if "eq" not in parts and "pe" not in parts:
                continue
            ohp = ohp_pool.tile([P, npj, NC, 2], f16, tag="ohp")
            if "eq" not in parts and "pe" in parts:
                nc.gpsimd.memset(ohp[:], 0)
            if "eq" in parts:
                nc.vector.tensor_tensor(
                    ohp[:],
                    predt[:, :, w0:w0 + wsub]
                        .rearrange("p c (q w) -> p q c w", w=2),
                    mxt[:, w0:w0 + wsub]
                        .rearrange("p (q w) -> p q w", w=2)
                        .unsqueeze(2).broadcast_to([P, npj, NC, 2]),
                    op=eq)
            if "pe" in parts:
                # per pair: lhsT [P, (c w)=100], rhs [P, (c w)=102];
                # host decodes the stride-2 diagonal blocks of psum[100,102]
                for q in range(npj):
                    nc.tensor.matmul(
                        psum_t[:],
                        lhsT=ohp[:, q, :, :].rearrange("p c w -> p (c w)"),
                        rhs=ohg[:, j * npj + q, :, :]
                            .rearrange("p c w -> p (c w)"),
                        start=first_mm[0], stop=False)
                    first_mm[0] = False

    if "pe" in parts:
        # closing matmul on zeroed operands to set stop
        zt = const_pool.tile([P, MD * NC], f16)
        zg = const_pool.tile([P, MD * C], f16)
        nc.vector.memset(zt[:], 0.0)
        nc.vector.memset(zg[:], 0.0)
        nc.tensor.matmul(psum_t[:], lhsT=zt[:], rhs=zg[:],
                         start=False, stop=True)
        histb = out_pool.tile([MD * NC, MD * C], f32)
        nc.scalar.copy(histb[:], psum_t[:])
        nc.sync.dma_start(hist_ap[:], histb[:])


def _build(repeat=None, internal_io=False, parts=ALL_PARTS):
    """repeat=None: production build (external pred/gt).
    repeat=R with internal_io=True: timing build — pred/gt are internal DRAM
    scratch (no host transfer), whole computation looped R times in-NEFF."""
    if "dve" in parts:  # back-compat alias
        parts = tuple(set(parts) - {"dve"}) + ("tree", "eq", "ohg")
    MD = 2 if MM_PAIR else 1
    nc = bacc.Bacc("TRN2", target_bir_lowering=False, debug=False,
                   num_devices=N_CORES)
    if internal_io:
        nc.dram_tensor("tick", [1], f32, kind="ExternalInput").ap()
        pred_ap = nc.dram_tensor("predt_i", [NC * MAIN], f16).ap()
        gt_ap = nc.dram_tensor("gt_i", [MAIN], f16).ap()
    else:
        pred_ap = nc.dram_tensor("predt", [NC * MAIN], f16,
                                 kind="ExternalInput").ap()
        gt_ap = nc.dram_tensor("gt", [MAIN], f16, kind="ExternalInput").ap()
    hist_ap = nc.dram_tensor("hist", [MD * NC, MD * C], f32,
                             kind="ExternalOutput").ap()

    pred_chunks = []
    gt_chunks = []
    so = 0   # sample offset
    for k, W in enumerate(W_SIZES):
        pred_chunks.append(
            pred_ap[NC * so:NC * (so + P * W)]
            .rearrange("(p c w) -> p c w", p=P, c=NC))
        gt_chunks.append(
            gt_ap[so:so + P * W].rearrange("(p w) -> p w", p=P))
        so += P * W

    with tile.TileContext(nc) as tc:
        with ExitStack() as ctx:
            bias_t = _emit_setup(nc, tc, ctx)
            if repeat is None:
                _emit_histogram(nc, tc, ctx, pred_chunks, gt_chunks, hist_ap,
                                bias_t, parts=parts)
            else:
                with tc.For_i(0, repeat, 1,
                              hint_engines=(mybir.EngineType.PE,
                                            mybir.EngineType.DVE)):
                    _emit_histogram(nc, tc, ctx, pred_chunks, gt_chunks,
                                    hist_ap, bias_t, parts=parts)
    nc.compile()
    return nc


def _get_nc():
    if "nc" not in _CACHE:
        _CACHE["nc"] = _build()
    return _CACHE["nc"]


def _pack_core(pred_core_f16: np.ndarray) -> np.ndarray:
    """[MAIN, 50] f16 -> packed flat [(chunk) p c w] layout."""
    out = []
    off = 0
    for W in W_SIZES:
        seg = pred_core_f16[off:off + P * W]
        out.append(np.ascontiguousarray(
            seg.reshape(P, W, NC).transpose(0, 2, 1)).reshape(-1))
        off += P * W
    return np.concatenate(out)


def _device_histogram(pred: np.ndarray, gt: np.ndarray,
                      want_trace: bool = False):
    """Run the SPMD kernel; return (global [51,51] f32 histogram, results).
    Device covers the first MAIN samples of each core's shard; the caller
    adds the host-side tail."""
    nc = _get_nc()
    in_maps = []
    for i in range(N_CORES):
        sl = slice(i * S_CORE, i * S_CORE + MAIN)
        predt = _pack_core(pred[sl, 1:C].astype(np.float16))
        gts = gt[sl].astype(np.float16)
        in_maps.append({"predt": predt, "gt": gts})
    res = run_bass_kernel_spmd(nc, in_maps, list(range(N_CORES)),
                               trace=want_trace)
    hist = np.zeros((C, C), dtype=np.float32)
    for r in res.results:
        hb = r["hist"]
        if MM_PAIR:
            hb4 = hb.reshape(NC, 2, C, 2)
            hist[1:C, :] += hb4[:, 0, :, 0] + hb4[:, 1, :, 1]
        else:
            hist[1:C, :] += hb
    return hist, res


def kernel(pred, rel_count, gt, istrain):
    pred = np.asarray(pred)
    rel_count = np.asarray(rel_count, dtype=np.float32)
    gt = np.asarray(gt)
    if not int(np.asarray(istrain)):
        return rel_count

    num = pred.shape[0]
    hist, _ = _device_histogram(pred, gt)

    # host tail: 72 samples per core, exact f32 argmax
    for i in range(N_CORES):
        lo = i * S_CORE + MAIN
        hi = (i + 1) * S_CORE
        pidx = np.argmax(pred[lo:hi, 1:C], axis=-1) + 1
        np.add.at(hist, (pidx, gt[lo:hi].astype(np.int64)), np.float32(1.0))

    # Small [51,51] postprocessing (exact mirror of the reference, f32).
    idx = hist.sum(axis=1, dtype=np.float32) / np.float32(num)
    gate = np.where(idx > 0.0, np.float32(0.9), np.float32(1.0))
    hist = hist.copy()
    hist[:, 0] = 0.0
    norm = hist / (hist.sum(axis=1, keepdims=True, dtype=np.float32)
                   + np.float32(1e-10))
    norm = norm.astype(np.float32)
    ema = gate[:, None] * rel_count + np.float32(0.1) * norm
    out = np.where(rel_count.sum(dtype=np.float32) == 0.0, norm, ema)
    return out.astype(np.float32)
